# revision 33
# baseline (speedup 1.0000x reference)
"""Causal self-attention (lit-gpt style, partial RoPE) on 8 Trainium2 NeuronCores.

Sharding: tensor-parallel over heads. Each core owns 4 of the 32 heads
end-to-end (QKV projection, RoPE, causal SDPA, and the K-split slice of the
output projection). Each core emits a partial (T, 4096) output; the host sums
the 8 partials (mathematically the all-reduce) and applies the bias folds.

Device-side compute dtype: bf16 matmul inputs with fp32 PSUM accumulation.

Model shapes (hardcoded): B=1, T=2048, C=4096, H=32, D=128, R=32 (rope),
rope base 10000.
"""

import sys
from contextlib import ExitStack

sys.path.insert(0, "/opt/trn_rl_repo")

import numpy as np
import ml_dtypes

import concourse.bass as bass
import concourse.bacc as bacc
import concourse.tile as tile
from concourse import mybir
from concourse import bass_utils

BF16 = ml_dtypes.bfloat16

T = 2048
C = 4096
H = 32
D = 128
R = 32
ROPE_BASE = 10000.0
N_CORES = 8
H_LOC = H // N_CORES          # 4 heads per core
KT = C // 128                 # 32 contraction tiles
NCH = T // 512                # 4 token chunks of 512
FQK = 2 * H_LOC               # 8 q/k feature tiles: f=2h -> q_h, f=2h+1 -> k_h
SCALE = 1.0 / float(np.sqrt(D))

# set by test.py to capture an NTFF profile; harness leaves False
TRACE = False
LAST_EXEC_NS = None
LAST_RESULTS = None

_CACHE = {}


def _build_program():
    """Build + compile the single-program SPMD Bass module (same code on all
    8 cores; per-core weights arrive via in_maps)."""
    nc = bacc.Bacc(
        "TRN2",
        target_bir_lowering=False,
        debug=False,
        enable_asserts=False,
        num_devices=N_CORES,
    )
    bf = mybir.dt.bfloat16
    f32 = mybir.dt.float32

    xT_d = nc.dram_tensor("xT", (C, T), bf, kind="ExternalInput").ap()
    wqkT_d = nc.dram_tensor("wqkT", (C, FQK * 128), bf, kind="ExternalInput").ap()
    wvT_d = nc.dram_tensor("wvT", (C, H_LOC * 128), bf, kind="ExternalInput").ap()
    wpT_d = nc.dram_tensor("wpT", (H_LOC * 128, C), bf, kind="ExternalInput").ap()
    bqk_d = nc.dram_tensor("bqk", (128, FQK), f32, kind="ExternalInput").ap()
    cos_d = nc.dram_tensor("cosP", (R, T), bf, kind="ExternalInput").ap()
    sin_d = nc.dram_tensor("sinP", (R, T), bf, kind="ExternalInput").ap()
    mask_d = nc.dram_tensor("maskP", (4, 128, 512), bf, kind="ExternalInput").ap()
    out_d = nc.dram_tensor("out", (T, C), bf, kind="ExternalOutput").ap()

    with tile.TileContext(nc) as tc:
        _emit(nc, tc, xT_d, wqkT_d, wvT_d, wpT_d, bqk_d, cos_d, sin_d, mask_d, out_d)

    nc.compile()
    return nc


def _emit(nc, tc, xT_d, wqkT_d, wvT_d, wpT_d, bqk_d, cos_d, sin_d, mask_d, out_d):
    bf = mybir.dt.bfloat16
    f32 = mybir.dt.float32

    # ---- persistent SBUF tensors (created before the working pools so that
    # pool release order stays LIFO: pools close first, singles after) -------
    frees = []

    def single(shape, dtype, name):
        t, fr = tc.tile(shape, dtype, name=name)
        frees.append(fr)
        return t

    qkT = single([128, FQK, T], bf, "qkT")          # Q^T/K^T: [d, f, t]
    vN = single([128, T // 128, H_LOC * 128], bf, "vN")  # V: [t%128, t//128, dv]
    yT = single([128, H_LOC, T], bf, "yT")          # normalized O^T per head
    ones = single([128, 128], bf, "ones")
    nc.vector.memset(ones, 1.0)
    bqk_sb = single([128, FQK], f32, "bqk_sb")
    nc.sync.dma_start(out=bqk_sb, in_=bqk_d)
    cos_sb = single([R, T], bf, "cos_sb")
    sin_sb = single([R, T], bf, "sin_sb")
    nc.sync.dma_start(out=cos_sb, in_=cos_d)
    nc.sync.dma_start(out=sin_sb, in_=sin_d)
    # diagonal causal mask tiles: mask_r[jj, ii] = 1.0 if ii >= jj + 128*r
    maskt = single([128, 4, 512], bf, "maskt")
    for r in range(4):
        nc.sync.dma_start(out=maskt[:, r, :], in_=mask_d[r])
    masks = [maskt[:, r, :] for r in range(4)]
    # v weights are chunk-independent: keep resident instead of re-streaming
    wv_res = single([128, KT, 512], bf, "wv_res")

    with ExitStack() as ctx:
        xqpool = ctx.enter_context(tc.tile_pool(name="xq", bufs=8))
        xvpool = ctx.enter_context(tc.tile_pool(name="xv", bufs=8))
        wqkpool = ctx.enter_context(tc.tile_pool(name="wqk", bufs=16))
        attpool = ctx.enter_context(tc.tile_pool(name="att", bufs=8))
        ropepool = ctx.enter_context(tc.tile_pool(name="rope", bufs=4))
        recippool = ctx.enter_context(tc.tile_pool(name="recip", bufs=2))
        stagepool = ctx.enter_context(tc.tile_pool(name="stage", bufs=4))
        wppool = ctx.enter_context(tc.tile_pool(name="wp", bufs=4))
        psum = ctx.enter_context(tc.tile_pool(name="psum", bufs=8, space="PSUM"))

        # PE warm-up: ~6us of throwaway matmuls issued while the first weight
        # and activation DMAs are still in flight. Gets the PE HAM clock gate
        # to 8/8 before the real work arrives, for free.
        warm = psum.tile([128, 128], f32, name="warm", tag="ps")
        for _ in range(96):
            nc.tensor.matmul(warm, lhsT=ones, rhs=ones, start=True, stop=True)

        def emit_rope(f, c):
            # q'[0:16]  = q[0:16]*cos - q[16:32]*sin
            # q'[16:32] = q[16:32]*cos + q[0:16]*sin
            # sin_sb rows 0..15 hold -sin, rows 16..31 hold +sin (host-folded).
            # Applied chunk-wise right after the chunk's q/k eviction so the
            # DVE work rides inside the QKV phase instead of gating SDPA.
            cs = slice(c * 512, (c + 1) * 512)
            rows = qkT[0:R, f, cs]
            swap = ropepool.tile([R, 512], bf, name=f"swap{f}_{c}", tag="swap")
            # gpsimd (SWDGE) queues: keeps these small SBUF->SBUF copies off
            # the sync queues that stream the main weight/activation tiles
            nc.gpsimd.dma_start(out=swap[0:16, :], in_=qkT[16:32, f, cs])
            nc.gpsimd.dma_start(out=swap[16:32, :], in_=qkT[0:16, f, cs])
            t1 = ropepool.tile([R, 512], bf, name=f"t1_{f}_{c}", tag="t1")
            nc.vector.tensor_mul(t1, swap, sin_sb[:, cs])
            nc.vector.tensor_mul(rows, rows, cos_sb[:, cs])
            nc.vector.tensor_add(rows, rows, t1)

        # ---- phase 1: QKV projection (+ RoPE overlapped with last v pass) --
        def emit_qk(c):
            # q/k in transposed layout (feature-major): 8 accumulator groups
            pss = [
                psum.tile([128, 512], f32, name=f"qk_ps{c}_{i}", tag="ps")
                for i in range(FQK)
            ]
            for kt in range(KT):
                xq = xqpool.tile([128, 512], bf, name=f"xq{c}_{kt}", tag="xq")
                nc.sync.dma_start(
                    out=xq,
                    in_=xT_d[kt * 128 : (kt + 1) * 128, c * 512 : (c + 1) * 512],
                )
                w8 = wqkpool.tile([128, FQK * 128], bf, name=f"w8_{c}_{kt}", tag="w8")
                nc.sync.dma_start(out=w8, in_=wqkT_d[kt * 128 : (kt + 1) * 128, :])
                for f in range(FQK):
                    nc.tensor.matmul(
                        pss[f],
                        lhsT=w8[:, f * 128 : (f + 1) * 128],
                        rhs=xq,
                        start=(kt == 0),
                        stop=(kt == KT - 1),
                    )
            for f in range(FQK):
                nc.vector.tensor_scalar_add(
                    qkT[:, f, c * 512 : (c + 1) * 512],
                    pss[f],
                    bqk_sb[:, f : f + 1],
                )

        def emit_v(c):
            # v in natural layout (token-major)
            if c == 0:
                for kt in range(KT):
                    nc.sync.dma_start(
                        out=wv_res[:, kt, :], in_=wvT_d[kt * 128 : (kt + 1) * 128, :]
                    )
            psv = [
                psum.tile([128, 512], f32, name=f"v_ps{c}_{i}", tag="ps")
                for i in range(4)
            ]
            for kt in range(KT):
                xv = xvpool.tile([128, 512], bf, name=f"xv{c}_{kt}", tag="xv")
                nc.sync.dma_start(
                    out=xv,
                    in_=xT_d[kt * 128 : (kt + 1) * 128, c * 512 : (c + 1) * 512],
                )
                for tt in range(4):
                    nc.tensor.matmul(
                        psv[tt],
                        lhsT=xv[:, tt * 128 : (tt + 1) * 128],
                        rhs=wv_res[:, kt, :],
                        start=(kt == 0),
                        stop=(kt == KT - 1),
                    )
            for tt in range(4):
                nc.vector.tensor_copy(vN[:, c * 4 + tt, :], psv[tt])

        # rope after v: its DVE ops must not queue ahead of the v-pass
        # evictions that release the PSUM slots the next chunk's qk needs
        for c in range(NCH):
            emit_qk(c)
            emit_v(c)
            for f in range(FQK):
                emit_rope(f, c)

        # ---- phase 3+4: causal SDPA (chunk-outer, head-inner) with the
        # output projection for each finished chunk interleaved, keeping PE
        # fed while the scalar engine works on the next chunk's exps --------
        def proj_block(c, nchs):
            # output projection for the token tiles of chunk c, nch columns
            for nch in nchs:
                wp = wppool.tile([128, H_LOC, 512], bf, name=f"wp{c}_{nch}", tag="wp")
                for h in range(H_LOC):
                    nc.sync.dma_start(
                        out=wp[:, h, :],
                        in_=wpT_d[
                            h * 128 : (h + 1) * 128, nch * 512 : (nch + 1) * 512
                        ],
                    )
                for tl in range(4):
                    tt = c * 4 + tl
                    pp = psum.tile([128, 512], f32, name=f"pp{c}_{nch}_{tl}", tag="ps")
                    for h in range(H_LOC):
                        nc.tensor.matmul(
                            pp,
                            lhsT=yT[:, h, tt * 128 : (tt + 1) * 128],
                            rhs=wp[:, h, :],
                            start=(h == 0),
                            stop=(h == H_LOC - 1),
                        )
                    st = stagepool.tile(
                        [128, 512], bf, name=f"st{c}_{nch}_{tl}", tag="st"
                    )
                    nc.vector.tensor_copy(st, pp)
                    nc.sync.dma_start(
                        out=out_d[
                            tt * 128 : (tt + 1) * 128, nch * 512 : (nch + 1) * 512
                        ],
                        in_=st,
                    )

        for c in range(NCH):
            njt = 4 * (c + 1)  # causal: key tiles 0 .. 4c+3
            for hp in range(H_LOC // 2):
                # previous chunk's projection matmuls interleave with this
                # chunk's exp-chain-bound SDPA to keep the PE stream dense
                if c > 0:
                    proj_block(c - 1, range(4 * hp, 4 * hp + 4))
                hh = (2 * hp, 2 * hp + 1)
                o_ps = {
                    h: psum.tile([128, 512], f32, name=f"o_ps{h}_{c}", tag="ps")
                    for h in hh
                }
                d_ps = {
                    h: psum.tile([128, 512], f32, name=f"d_ps{h}_{c}", tag="ps")
                    for h in hh
                }
                # interleave two heads so PE always has an independent matmul
                # while the scalar engine works on the other head's exp
                for jt in range(njt):
                    for h in hh:
                        s_ps = psum.tile(
                            [128, 512], f32, name=f"s_ps{h}_{c}_{jt}", tag="ps"
                        )
                        nc.tensor.matmul(
                            s_ps,
                            lhsT=qkT[:, 2 * h + 1, jt * 128 : (jt + 1) * 128],
                            rhs=qkT[:, 2 * h, c * 512 : (c + 1) * 512],
                            start=True,
                            stop=True,
                        )
                        att = attpool.tile(
                            [128, 512], bf, name=f"att{h}_{c}_{jt}", tag="att"
                        )
                        nc.scalar.activation(
                            out=att,
                            in_=s_ps,
                            func=mybir.ActivationFunctionType.Exp,
                            scale=SCALE,
                        )
                        r = jt - 4 * c
                        if r >= 0:
                            nc.vector.tensor_mul(att, att, masks[r])
                        nc.tensor.matmul(
                            d_ps[h],
                            lhsT=ones,
                            rhs=att,
                            start=(jt == 0),
                            stop=(jt == njt - 1),
                        )
                        nc.tensor.matmul(
                            o_ps[h],
                            lhsT=vN[:, jt, h * 128 : (h + 1) * 128],
                            rhs=att,
                            start=(jt == 0),
                            stop=(jt == njt - 1),
                        )
                for h in hh:
                    rec = recippool.tile([128, 512], f32, name=f"rec{h}_{c}", tag="rec")
                    nc.vector.reciprocal(rec, d_ps[h])
                    nc.vector.tensor_mul(yT[:, h, c * 512 : (c + 1) * 512], o_ps[h], rec)
        proj_block(NCH - 1, range(C // 512))

    for fr in reversed(frees):
        fr()


def _rope_tables():
    theta = 1.0 / (ROPE_BASE ** (np.arange(0, R, 2, dtype=np.float64) / R))  # (16,)
    ang = np.outer(np.arange(T, dtype=np.float64), theta)  # (T, 16)
    cos = np.cos(ang).T  # (16, T)
    sin = np.sin(ang).T
    cosP = np.concatenate([cos, cos], axis=0)  # (32, T)
    sinP = np.concatenate([-sin, sin], axis=0)
    return np.ascontiguousarray(cosP).astype(BF16), np.ascontiguousarray(sinP).astype(BF16)


def kernel(x, w_attn, b_attn, w_proj, b_proj):
    x = np.asarray(x, dtype=np.float32)
    w_attn = np.asarray(w_attn, dtype=np.float32)
    b_attn = np.asarray(b_attn, dtype=np.float32)
    w_proj = np.asarray(w_proj, dtype=np.float32)
    b_proj = np.asarray(b_proj, dtype=np.float32)
    B = x.shape[0]
    assert (B, x.shape[1], x.shape[2]) == (1, T, C)

    if "nc" not in _CACHE:
        _CACHE["nc"] = _build_program()
    nc = _CACHE["nc"]

    xT = np.ascontiguousarray(x[0].T).astype(BF16)  # (C, T)
    cosP, sinP = _rope_tables()
    # diagonal causal mask tiles: maskP[r, jj, ii] = 1.0 iff ii >= jj + 128*r
    jj = np.arange(128)[None, :, None]
    ii = np.arange(512)[None, None, :]
    rr = (128 * np.arange(4))[:, None, None]
    maskP = (ii >= jj + rr).astype(BF16)  # (4, 128, 512)

    # w_attn rows per head h: [q (128), k (128), v (128)] at offset h*384
    wa = w_attn.reshape(H, 3, D, C)
    ba = b_attn.reshape(H, 3, D)
    in_maps = []
    for core in range(N_CORES):
        hs = range(core * H_LOC, (core + 1) * H_LOC)
        qk_rows = np.concatenate(
            [wa[h, t] for h in hs for t in (0, 1)], axis=0
        )  # (1024, C)  order: q_h0, k_h0, q_h1, k_h1, ...
        v_rows = np.concatenate([wa[h, 2] for h in hs], axis=0)  # (512, C)
        wqkT = np.ascontiguousarray(qk_rows.T).astype(BF16)  # (C, 1024)
        wvT = np.ascontiguousarray(v_rows.T).astype(BF16)  # (C, 512)
        wpT = np.ascontiguousarray(
            w_proj[:, core * 512 : (core + 1) * 512].T
        ).astype(BF16)  # (512, C)
        bqk = np.ascontiguousarray(
            np.stack([ba[h, t] for h in hs for t in (0, 1)], axis=0).T
        ).astype(np.float32)  # (128, 8)
        in_maps.append(
            dict(
                xT=xT, wqkT=wqkT, wvT=wvT, wpT=wpT, bqk=bqk,
                cosP=cosP, sinP=sinP, maskP=maskP,
            )
        )

    res = bass_utils.run_bass_kernel_spmd(
        nc, in_maps, core_ids=list(range(N_CORES)), trace=TRACE
    )
    global LAST_EXEC_NS, LAST_RESULTS
    LAST_EXEC_NS = res.exec_time_ns
    LAST_RESULTS = res

    out = np.zeros((T, C), dtype=np.float32)
    for core in range(N_CORES):
        out += res.results[core]["out"]

    # bias folds: q/k biases were applied on device; the v bias adds exactly
    # b_v to every y row (softmax rows sum to 1), so it folds into the output
    # bias along with b_proj.
    b_v = ba[:, 2, :].reshape(-1)  # (4096,)
    out += (w_proj @ b_v + b_proj)[None, :]
    return out.reshape(B, T, C).astype(np.float32)


# revision 34
# speedup vs baseline: 1.0068x; 1.0068x over previous
"""Causal self-attention (lit-gpt style, partial RoPE) on 8 Trainium2 NeuronCores.

Sharding: tensor-parallel over heads. Each core owns 4 of the 32 heads
end-to-end (QKV projection, RoPE, causal SDPA, and the K-split slice of the
output projection). Each core emits a partial (T, 4096) output; the host sums
the 8 partials (mathematically the all-reduce) and applies the bias folds.

Device-side compute dtype: bf16 matmul inputs with fp32 PSUM accumulation.

Model shapes (hardcoded): B=1, T=2048, C=4096, H=32, D=128, R=32 (rope),
rope base 10000.
"""

import sys
from contextlib import ExitStack

sys.path.insert(0, "/opt/trn_rl_repo")

import numpy as np
import ml_dtypes

import concourse.bass as bass
import concourse.bacc as bacc
import concourse.tile as tile
from concourse import mybir
from concourse import bass_utils

BF16 = ml_dtypes.bfloat16

T = 2048
C = 4096
H = 32
D = 128
R = 32
ROPE_BASE = 10000.0
N_CORES = 8
H_LOC = H // N_CORES          # 4 heads per core
KT = C // 128                 # 32 contraction tiles
NCH = T // 512                # 4 token chunks of 512
FQK = 2 * H_LOC               # 8 q/k feature tiles: f=2h -> q_h, f=2h+1 -> k_h
SCALE = 1.0 / float(np.sqrt(D))

# set by test.py to capture an NTFF profile; harness leaves False
TRACE = False
LAST_EXEC_NS = None
LAST_RESULTS = None

_CACHE = {}


def _build_program():
    """Build + compile the single-program SPMD Bass module (same code on all
    8 cores; per-core weights arrive via in_maps)."""
    nc = bacc.Bacc(
        "TRN2",
        target_bir_lowering=False,
        debug=False,
        enable_asserts=False,
        num_devices=N_CORES,
    )
    bf = mybir.dt.bfloat16
    f32 = mybir.dt.float32

    xT_d = nc.dram_tensor("xT", (C, T), bf, kind="ExternalInput").ap()
    wqkT_d = nc.dram_tensor("wqkT", (C, FQK * 128), bf, kind="ExternalInput").ap()
    wvT_d = nc.dram_tensor("wvT", (C, H_LOC * 128), bf, kind="ExternalInput").ap()
    wpT_d = nc.dram_tensor("wpT", (H_LOC * 128, C), bf, kind="ExternalInput").ap()
    bqk_d = nc.dram_tensor("bqk", (128, FQK), f32, kind="ExternalInput").ap()
    cos_d = nc.dram_tensor("cosP", (R, T), bf, kind="ExternalInput").ap()
    sin_d = nc.dram_tensor("sinP", (R, T), bf, kind="ExternalInput").ap()
    mask_d = nc.dram_tensor("maskP", (4, 128, 512), bf, kind="ExternalInput").ap()
    out_d = nc.dram_tensor("out", (T, C), bf, kind="ExternalOutput").ap()

    with tile.TileContext(nc) as tc:
        _emit(nc, tc, xT_d, wqkT_d, wvT_d, wpT_d, bqk_d, cos_d, sin_d, mask_d, out_d)

    nc.compile()
    return nc


def _emit(nc, tc, xT_d, wqkT_d, wvT_d, wpT_d, bqk_d, cos_d, sin_d, mask_d, out_d):
    bf = mybir.dt.bfloat16
    f32 = mybir.dt.float32

    # ---- persistent SBUF tensors (created before the working pools so that
    # pool release order stays LIFO: pools close first, singles after) -------
    frees = []

    def single(shape, dtype, name):
        t, fr = tc.tile(shape, dtype, name=name)
        frees.append(fr)
        return t

    qkT = single([128, FQK, T], bf, "qkT")          # Q^T/K^T: [d, f, t]
    vN = single([128, T // 128, H_LOC * 128], bf, "vN")  # V: [t%128, t//128, dv]
    yT = single([128, H_LOC, T], bf, "yT")          # normalized O^T per head
    ones = single([128, 128], bf, "ones")
    nc.vector.memset(ones, 1.0)
    bqk_sb = single([128, FQK], f32, "bqk_sb")
    nc.sync.dma_start(out=bqk_sb, in_=bqk_d)
    cos_sb = single([R, T], bf, "cos_sb")
    sin_sb = single([R, T], bf, "sin_sb")
    nc.sync.dma_start(out=cos_sb, in_=cos_d)
    nc.sync.dma_start(out=sin_sb, in_=sin_d)
    # diagonal causal mask tiles: mask_r[jj, ii] = 1.0 if ii >= jj + 128*r
    maskt = single([128, 4, 512], bf, "maskt")
    for r in range(4):
        nc.sync.dma_start(out=maskt[:, r, :], in_=mask_d[r])
    masks = [maskt[:, r, :] for r in range(4)]
    # v weights are chunk-independent: keep resident instead of re-streaming
    wv_res = single([128, KT, 512], bf, "wv_res")

    with ExitStack() as ctx:
        xqpool = ctx.enter_context(tc.tile_pool(name="xq", bufs=8))
        xvpool = ctx.enter_context(tc.tile_pool(name="xv", bufs=8))
        wqkpool = ctx.enter_context(tc.tile_pool(name="wqk", bufs=16))
        attpool = ctx.enter_context(tc.tile_pool(name="att", bufs=8))
        ropepool = ctx.enter_context(tc.tile_pool(name="rope", bufs=4))
        recippool = ctx.enter_context(tc.tile_pool(name="recip", bufs=2))
        stagepool = ctx.enter_context(tc.tile_pool(name="stage", bufs=4))
        wppool = ctx.enter_context(tc.tile_pool(name="wp", bufs=4))
        psum = ctx.enter_context(tc.tile_pool(name="psum", bufs=8, space="PSUM"))

        # PE warm-up: ~6us of throwaway matmuls issued while the first weight
        # and activation DMAs are still in flight. Gets the PE HAM clock gate
        # to 8/8 before the real work arrives, for free.
        warm = psum.tile([128, 128], f32, name="warm", tag="ps")
        for _ in range(96):
            nc.tensor.matmul(warm, lhsT=ones, rhs=ones, start=True, stop=True)

        def emit_rope(f, c):
            # q'[0:16]  = q[0:16]*cos - q[16:32]*sin
            # q'[16:32] = q[16:32]*cos + q[0:16]*sin
            # sin_sb rows 0..15 hold -sin, rows 16..31 hold +sin (host-folded).
            # Applied chunk-wise right after the chunk's q/k eviction so the
            # DVE work rides inside the QKV phase instead of gating SDPA.
            cs = slice(c * 512, (c + 1) * 512)
            rows = qkT[0:R, f, cs]
            swap = ropepool.tile([R, 512], bf, name=f"swap{f}_{c}", tag="swap")
            # gpsimd (SWDGE) queues: keeps these small SBUF->SBUF copies off
            # the sync queues that stream the main weight/activation tiles
            nc.gpsimd.dma_start(out=swap[0:16, :], in_=qkT[16:32, f, cs])
            nc.gpsimd.dma_start(out=swap[16:32, :], in_=qkT[0:16, f, cs])
            t1 = ropepool.tile([R, 512], bf, name=f"t1_{f}_{c}", tag="t1")
            nc.vector.tensor_mul(t1, swap, sin_sb[:, cs])
            nc.vector.tensor_mul(rows, rows, cos_sb[:, cs])
            nc.vector.tensor_add(rows, rows, t1)

        # ---- phase 1: QKV projection (+ RoPE overlapped with last v pass) --
        def emit_qk(c):
            # q/k in transposed layout (feature-major): 8 accumulator groups
            pss = [
                psum.tile([128, 512], f32, name=f"qk_ps{c}_{i}", tag="ps")
                for i in range(FQK)
            ]
            for kt in range(KT):
                xq = xqpool.tile([128, 512], bf, name=f"xq{c}_{kt}", tag="xq")
                nc.sync.dma_start(
                    out=xq,
                    in_=xT_d[kt * 128 : (kt + 1) * 128, c * 512 : (c + 1) * 512],
                )
                w8 = wqkpool.tile([128, FQK * 128], bf, name=f"w8_{c}_{kt}", tag="w8")
                nc.sync.dma_start(out=w8, in_=wqkT_d[kt * 128 : (kt + 1) * 128, :])
                for f in range(FQK):
                    nc.tensor.matmul(
                        pss[f],
                        lhsT=w8[:, f * 128 : (f + 1) * 128],
                        rhs=xq,
                        start=(kt == 0),
                        stop=(kt == KT - 1),
                    )
            for f in range(FQK):
                nc.vector.tensor_scalar_add(
                    qkT[:, f, c * 512 : (c + 1) * 512],
                    pss[f],
                    bqk_sb[:, f : f + 1],
                )

        def emit_v(c):
            # v in natural layout (token-major)
            if c == 0:
                for kt in range(KT):
                    nc.sync.dma_start(
                        out=wv_res[:, kt, :], in_=wvT_d[kt * 128 : (kt + 1) * 128, :]
                    )
            psv = [
                psum.tile([128, 512], f32, name=f"v_ps{c}_{i}", tag="ps")
                for i in range(4)
            ]
            for kt in range(KT):
                xv = xvpool.tile([128, 512], bf, name=f"xv{c}_{kt}", tag="xv")
                nc.sync.dma_start(
                    out=xv,
                    in_=xT_d[kt * 128 : (kt + 1) * 128, c * 512 : (c + 1) * 512],
                )
                for tt in range(4):
                    nc.tensor.matmul(
                        psv[tt],
                        lhsT=xv[:, tt * 128 : (tt + 1) * 128],
                        rhs=wv_res[:, kt, :],
                        start=(kt == 0),
                        stop=(kt == KT - 1),
                    )
            for tt in range(4):
                nc.vector.tensor_copy(vN[:, c * 4 + tt, :], psv[tt])

        # rope after v: its DVE ops must not queue ahead of the v-pass
        # evictions that release the PSUM slots the next chunk's qk needs
        for c in range(NCH):
            emit_qk(c)
            emit_v(c)
            for f in range(FQK):
                emit_rope(f, c)

        # ---- phase 3+4: causal SDPA (chunk-outer, head-inner) with the
        # output projection for each finished chunk interleaved, keeping PE
        # fed while the scalar engine works on the next chunk's exps --------
        def proj_block(c, nchs):
            # output projection for the token tiles of chunk c, nch columns
            for nch in nchs:
                wp = wppool.tile([128, H_LOC, 512], bf, name=f"wp{c}_{nch}", tag="wp")
                for h in range(H_LOC):
                    nc.sync.dma_start(
                        out=wp[:, h, :],
                        in_=wpT_d[
                            h * 128 : (h + 1) * 128, nch * 512 : (nch + 1) * 512
                        ],
                    )
                for tl in range(4):
                    tt = c * 4 + tl
                    pp = psum.tile([128, 512], f32, name=f"pp{c}_{nch}_{tl}", tag="ps")
                    for h in range(H_LOC):
                        nc.tensor.matmul(
                            pp,
                            lhsT=yT[:, h, tt * 128 : (tt + 1) * 128],
                            rhs=wp[:, h, :],
                            start=(h == 0),
                            stop=(h == H_LOC - 1),
                        )
                    st = stagepool.tile(
                        [128, 512], bf, name=f"st{c}_{nch}_{tl}", tag="st"
                    )
                    # alternate eviction between DVE and ACT: keeps the DVE
                    # FIFO short so the SDPA-critical mask/normalize ops
                    # behind it aren't delayed by bulk projection copies
                    if tl % 2 == 0:
                        nc.vector.tensor_copy(st, pp)
                    else:
                        nc.scalar.copy(st, pp)
                    nc.sync.dma_start(
                        out=out_d[
                            tt * 128 : (tt + 1) * 128, nch * 512 : (nch + 1) * 512
                        ],
                        in_=st,
                    )

        for c in range(NCH):
            njt = 4 * (c + 1)  # causal: key tiles 0 .. 4c+3
            for hp in range(H_LOC // 2):
                # previous chunk's projection matmuls interleave with this
                # chunk's exp-chain-bound SDPA to keep the PE stream dense
                if c > 0:
                    proj_block(c - 1, range(4 * hp, 4 * hp + 4))
                hh = (2 * hp, 2 * hp + 1)
                o_ps = {
                    h: psum.tile([128, 512], f32, name=f"o_ps{h}_{c}", tag="ps")
                    for h in hh
                }
                d_ps = {
                    h: psum.tile([128, 512], f32, name=f"d_ps{h}_{c}", tag="ps")
                    for h in hh
                }
                # interleave two heads so PE always has an independent matmul
                # while the scalar engine works on the other head's exp
                for jt in range(njt):
                    for h in hh:
                        s_ps = psum.tile(
                            [128, 512], f32, name=f"s_ps{h}_{c}_{jt}", tag="ps"
                        )
                        nc.tensor.matmul(
                            s_ps,
                            lhsT=qkT[:, 2 * h + 1, jt * 128 : (jt + 1) * 128],
                            rhs=qkT[:, 2 * h, c * 512 : (c + 1) * 512],
                            start=True,
                            stop=True,
                        )
                        att = attpool.tile(
                            [128, 512], bf, name=f"att{h}_{c}_{jt}", tag="att"
                        )
                        nc.scalar.activation(
                            out=att,
                            in_=s_ps,
                            func=mybir.ActivationFunctionType.Exp,
                            scale=SCALE,
                        )
                        r = jt - 4 * c
                        if r >= 0:
                            nc.vector.tensor_mul(att, att, masks[r])
                        nc.tensor.matmul(
                            d_ps[h],
                            lhsT=ones,
                            rhs=att,
                            start=(jt == 0),
                            stop=(jt == njt - 1),
                        )
                        nc.tensor.matmul(
                            o_ps[h],
                            lhsT=vN[:, jt, h * 128 : (h + 1) * 128],
                            rhs=att,
                            start=(jt == 0),
                            stop=(jt == njt - 1),
                        )
                for h in hh:
                    rec = recippool.tile([128, 512], f32, name=f"rec{h}_{c}", tag="rec")
                    nc.vector.reciprocal(rec, d_ps[h])
                    nc.vector.tensor_mul(yT[:, h, c * 512 : (c + 1) * 512], o_ps[h], rec)
        proj_block(NCH - 1, range(C // 512))

    for fr in reversed(frees):
        fr()


def _rope_tables():
    theta = 1.0 / (ROPE_BASE ** (np.arange(0, R, 2, dtype=np.float64) / R))  # (16,)
    ang = np.outer(np.arange(T, dtype=np.float64), theta)  # (T, 16)
    cos = np.cos(ang).T  # (16, T)
    sin = np.sin(ang).T
    cosP = np.concatenate([cos, cos], axis=0)  # (32, T)
    sinP = np.concatenate([-sin, sin], axis=0)
    return np.ascontiguousarray(cosP).astype(BF16), np.ascontiguousarray(sinP).astype(BF16)


def kernel(x, w_attn, b_attn, w_proj, b_proj):
    x = np.asarray(x, dtype=np.float32)
    w_attn = np.asarray(w_attn, dtype=np.float32)
    b_attn = np.asarray(b_attn, dtype=np.float32)
    w_proj = np.asarray(w_proj, dtype=np.float32)
    b_proj = np.asarray(b_proj, dtype=np.float32)
    B = x.shape[0]
    assert (B, x.shape[1], x.shape[2]) == (1, T, C)

    if "nc" not in _CACHE:
        _CACHE["nc"] = _build_program()
    nc = _CACHE["nc"]

    xT = np.ascontiguousarray(x[0].T).astype(BF16)  # (C, T)
    cosP, sinP = _rope_tables()
    # diagonal causal mask tiles: maskP[r, jj, ii] = 1.0 iff ii >= jj + 128*r
    jj = np.arange(128)[None, :, None]
    ii = np.arange(512)[None, None, :]
    rr = (128 * np.arange(4))[:, None, None]
    maskP = (ii >= jj + rr).astype(BF16)  # (4, 128, 512)

    # w_attn rows per head h: [q (128), k (128), v (128)] at offset h*384
    wa = w_attn.reshape(H, 3, D, C)
    ba = b_attn.reshape(H, 3, D)
    in_maps = []
    for core in range(N_CORES):
        hs = range(core * H_LOC, (core + 1) * H_LOC)
        qk_rows = np.concatenate(
            [wa[h, t] for h in hs for t in (0, 1)], axis=0
        )  # (1024, C)  order: q_h0, k_h0, q_h1, k_h1, ...
        v_rows = np.concatenate([wa[h, 2] for h in hs], axis=0)  # (512, C)
        wqkT = np.ascontiguousarray(qk_rows.T).astype(BF16)  # (C, 1024)
        wvT = np.ascontiguousarray(v_rows.T).astype(BF16)  # (C, 512)
        wpT = np.ascontiguousarray(
            w_proj[:, core * 512 : (core + 1) * 512].T
        ).astype(BF16)  # (512, C)
        bqk = np.ascontiguousarray(
            np.stack([ba[h, t] for h in hs for t in (0, 1)], axis=0).T
        ).astype(np.float32)  # (128, 8)
        in_maps.append(
            dict(
                xT=xT, wqkT=wqkT, wvT=wvT, wpT=wpT, bqk=bqk,
                cosP=cosP, sinP=sinP, maskP=maskP,
            )
        )

    res = bass_utils.run_bass_kernel_spmd(
        nc, in_maps, core_ids=list(range(N_CORES)), trace=TRACE
    )
    global LAST_EXEC_NS, LAST_RESULTS
    LAST_EXEC_NS = res.exec_time_ns
    LAST_RESULTS = res

    out = np.zeros((T, C), dtype=np.float32)
    for core in range(N_CORES):
        out += res.results[core]["out"]

    # bias folds: q/k biases were applied on device; the v bias adds exactly
    # b_v to every y row (softmax rows sum to 1), so it folds into the output
    # bias along with b_proj.
    b_v = ba[:, 2, :].reshape(-1)  # (4096,)
    out += (w_proj @ b_v + b_proj)[None, :]
    return out.reshape(B, T, C).astype(np.float32)


# revision 36
# speedup vs baseline: 1.0414x; 1.0344x over previous
"""Causal self-attention (lit-gpt style, partial RoPE) on 8 Trainium2 NeuronCores.

Sharding: tensor-parallel over heads. Each core owns 4 of the 32 heads
end-to-end (QKV projection, RoPE, causal SDPA, and the K-split slice of the
output projection). Each core emits a partial (T, 4096) output; the host sums
the 8 partials (mathematically the all-reduce) and applies the bias folds.

Device-side compute dtype: bf16 matmul inputs with fp32 PSUM accumulation.

Model shapes (hardcoded): B=1, T=2048, C=4096, H=32, D=128, R=32 (rope),
rope base 10000.
"""

import sys
from contextlib import ExitStack

sys.path.insert(0, "/opt/trn_rl_repo")

import numpy as np
import ml_dtypes

import concourse.bass as bass
import concourse.bacc as bacc
import concourse.tile as tile
from concourse import mybir
from concourse import bass_utils

BF16 = ml_dtypes.bfloat16

T = 2048
C = 4096
H = 32
D = 128
R = 32
ROPE_BASE = 10000.0
N_CORES = 8
H_LOC = H // N_CORES          # 4 heads per core
KT = C // 128                 # 32 contraction tiles
NCH = T // 512                # 4 token chunks of 512
FQK = 2 * H_LOC               # 8 q/k feature tiles: f=2h -> q_h, f=2h+1 -> k_h
SCALE = 1.0 / float(np.sqrt(D))

# set by test.py to capture an NTFF profile; harness leaves False
TRACE = False
LAST_EXEC_NS = None
LAST_RESULTS = None

_CACHE = {}


def _build_program():
    """Build + compile the single-program SPMD Bass module (same code on all
    8 cores; per-core weights arrive via in_maps)."""
    nc = bacc.Bacc(
        "TRN2",
        target_bir_lowering=False,
        debug=False,
        enable_asserts=False,
        num_devices=N_CORES,
    )
    bf = mybir.dt.bfloat16
    f32 = mybir.dt.float32

    xT_d = nc.dram_tensor("xT", (C, T), bf, kind="ExternalInput").ap()
    wqkT_d = nc.dram_tensor("wqkT", (C, FQK * 128), bf, kind="ExternalInput").ap()
    wvT_d = nc.dram_tensor("wvT", (C, H_LOC * 128), bf, kind="ExternalInput").ap()
    wpT_d = nc.dram_tensor("wpT", (H_LOC * 128, C), bf, kind="ExternalInput").ap()
    bqk_d = nc.dram_tensor("bqk", (128, FQK), f32, kind="ExternalInput").ap()
    cos_d = nc.dram_tensor("cosP", (R, T), bf, kind="ExternalInput").ap()
    sin_d = nc.dram_tensor("sinP", (R, T), bf, kind="ExternalInput").ap()
    mask_d = nc.dram_tensor("maskP", (4, 128, 512), bf, kind="ExternalInput").ap()
    out_d = nc.dram_tensor("out", (T, C), bf, kind="ExternalOutput").ap()

    with tile.TileContext(nc) as tc:
        _emit(nc, tc, xT_d, wqkT_d, wvT_d, wpT_d, bqk_d, cos_d, sin_d, mask_d, out_d)

    nc.compile()
    return nc


def _emit(nc, tc, xT_d, wqkT_d, wvT_d, wpT_d, bqk_d, cos_d, sin_d, mask_d, out_d):
    bf = mybir.dt.bfloat16
    f32 = mybir.dt.float32

    # ---- persistent SBUF tensors (created before the working pools so that
    # pool release order stays LIFO: pools close first, singles after) -------
    frees = []

    def single(shape, dtype, name):
        t, fr = tc.tile(shape, dtype, name=name)
        frees.append(fr)
        return t

    qkT = single([128, FQK, T], bf, "qkT")          # Q^T/K^T: [d, f, t]
    vN = single([128, T // 128, H_LOC * 128], bf, "vN")  # V: [t%128, t//128, dv]
    yT = single([128, H_LOC, T], bf, "yT")          # normalized O^T per head
    ones = single([128, 128], bf, "ones")
    nc.vector.memset(ones, 1.0)
    bqk_sb = single([128, FQK], f32, "bqk_sb")
    nc.sync.dma_start(out=bqk_sb, in_=bqk_d)
    cos_sb = single([R, T], bf, "cos_sb")
    sin_sb = single([R, T], bf, "sin_sb")
    nc.sync.dma_start(out=cos_sb, in_=cos_d)
    nc.sync.dma_start(out=sin_sb, in_=sin_d)
    # diagonal causal mask tiles: mask_r[jj, ii] = 1.0 if ii >= jj + 128*r
    maskt = single([128, 4, 512], bf, "maskt")
    for r in range(4):
        nc.sync.dma_start(out=maskt[:, r, :], in_=mask_d[r])
    masks = [maskt[:, r, :] for r in range(4)]
    # v weights are chunk-independent: keep resident instead of re-streaming
    wv_res = single([128, KT, 512], bf, "wv_res")

    with ExitStack() as ctx:
        xqpool = ctx.enter_context(tc.tile_pool(name="xq", bufs=8))
        xvpool = ctx.enter_context(tc.tile_pool(name="xv", bufs=8))
        wqkpool = ctx.enter_context(tc.tile_pool(name="wqk", bufs=16))
        attpool = ctx.enter_context(tc.tile_pool(name="att", bufs=8))
        ropepool = ctx.enter_context(tc.tile_pool(name="rope", bufs=4))
        recippool = ctx.enter_context(tc.tile_pool(name="recip", bufs=2))
        stagepool = ctx.enter_context(tc.tile_pool(name="stage", bufs=4))
        wppool = ctx.enter_context(tc.tile_pool(name="wp", bufs=4))
        psum = ctx.enter_context(tc.tile_pool(name="psum", bufs=8, space="PSUM"))

        # PE warm-up: ~6us of throwaway matmuls issued while the first weight
        # and activation DMAs are still in flight. Gets the PE HAM clock gate
        # to 8/8 before the real work arrives, for free.
        warm = psum.tile([128, 128], f32, name="warm", tag="ps")
        for _ in range(96):
            nc.tensor.matmul(warm, lhsT=ones, rhs=ones, start=True, stop=True)

        def emit_rope(f, c):
            # q'[0:16]  = q[0:16]*cos - q[16:32]*sin
            # q'[16:32] = q[16:32]*cos + q[0:16]*sin
            # sin_sb rows 0..15 hold -sin, rows 16..31 hold +sin (host-folded).
            # Applied chunk-wise right after the chunk's q/k eviction so the
            # DVE work rides inside the QKV phase instead of gating SDPA.
            cs = slice(c * 512, (c + 1) * 512)
            rows = qkT[0:R, f, cs]
            swap = ropepool.tile([R, 512], bf, name=f"swap{f}_{c}", tag="swap")
            # gpsimd (SWDGE) queues: keeps these small SBUF->SBUF copies off
            # the sync queues that stream the main weight/activation tiles
            nc.gpsimd.dma_start(out=swap[0:16, :], in_=qkT[16:32, f, cs])
            nc.gpsimd.dma_start(out=swap[16:32, :], in_=qkT[0:16, f, cs])
            t1 = ropepool.tile([R, 512], bf, name=f"t1_{f}_{c}", tag="t1")
            nc.vector.tensor_mul(t1, swap, sin_sb[:, cs])
            nc.vector.tensor_mul(rows, rows, cos_sb[:, cs])
            nc.vector.tensor_add(rows, rows, t1)

        # ---- phase 1: QKV projection (+ RoPE overlapped with last v pass) --
        def emit_qk(c):
            # q/k in transposed layout (feature-major): 8 accumulator groups
            pss = [
                psum.tile([128, 512], f32, name=f"qk_ps{c}_{i}", tag="ps")
                for i in range(FQK)
            ]
            for kt in range(KT):
                xq = xqpool.tile([128, 512], bf, name=f"xq{c}_{kt}", tag="xq")
                nc.sync.dma_start(
                    out=xq,
                    in_=xT_d[kt * 128 : (kt + 1) * 128, c * 512 : (c + 1) * 512],
                )
                w8 = wqkpool.tile([128, FQK * 128], bf, name=f"w8_{c}_{kt}", tag="w8")
                nc.sync.dma_start(out=w8, in_=wqkT_d[kt * 128 : (kt + 1) * 128, :])
                for f in range(FQK):
                    nc.tensor.matmul(
                        pss[f],
                        lhsT=w8[:, f * 128 : (f + 1) * 128],
                        rhs=xq,
                        start=(kt == 0),
                        stop=(kt == KT - 1),
                    )
            for f in range(FQK):
                nc.vector.tensor_scalar_add(
                    qkT[:, f, c * 512 : (c + 1) * 512],
                    pss[f],
                    bqk_sb[:, f : f + 1],
                )

        def emit_v(c):
            # v in natural layout (token-major)
            if c == 0:
                for kt in range(KT):
                    nc.sync.dma_start(
                        out=wv_res[:, kt, :], in_=wvT_d[kt * 128 : (kt + 1) * 128, :]
                    )
            psv = [
                psum.tile([128, 512], f32, name=f"v_ps{c}_{i}", tag="ps")
                for i in range(4)
            ]
            for kt in range(KT):
                xv = xvpool.tile([128, 512], bf, name=f"xv{c}_{kt}", tag="xv")
                nc.sync.dma_start(
                    out=xv,
                    in_=xT_d[kt * 128 : (kt + 1) * 128, c * 512 : (c + 1) * 512],
                )
                for tt in range(4):
                    nc.tensor.matmul(
                        psv[tt],
                        lhsT=xv[:, tt * 128 : (tt + 1) * 128],
                        rhs=wv_res[:, kt, :],
                        start=(kt == 0),
                        stop=(kt == KT - 1),
                    )
            for tt in range(4):
                nc.vector.tensor_copy(vN[:, c * 4 + tt, :], psv[tt])

        # rope after v: its DVE ops must not queue ahead of the v-pass
        # evictions that release the PSUM slots the next chunk's qk needs
        for c in range(NCH):
            emit_qk(c)
            emit_v(c)
            for f in range(FQK):
                emit_rope(f, c)

        # ---- phase 3+4: causal SDPA (chunk-outer, head-inner) with the
        # output projection for each finished chunk interleaved, keeping PE
        # fed while the scalar engine works on the next chunk's exps --------
        def proj_block(c, nchs):
            # output projection for the token tiles of chunk c, nch columns
            for nch in nchs:
                wp = wppool.tile([128, H_LOC, 512], bf, name=f"wp{c}_{nch}", tag="wp")
                for h in range(H_LOC):
                    nc.sync.dma_start(
                        out=wp[:, h, :],
                        in_=wpT_d[
                            h * 128 : (h + 1) * 128, nch * 512 : (nch + 1) * 512
                        ],
                    )
                for tl in range(4):
                    tt = c * 4 + tl
                    pp = psum.tile([128, 512], f32, name=f"pp{c}_{nch}_{tl}", tag="ps")
                    for h in range(H_LOC):
                        nc.tensor.matmul(
                            pp,
                            lhsT=yT[:, h, tt * 128 : (tt + 1) * 128],
                            rhs=wp[:, h, :],
                            start=(h == 0),
                            stop=(h == H_LOC - 1),
                        )
                    st = stagepool.tile(
                        [128, 512], bf, name=f"st{c}_{nch}_{tl}", tag="st"
                    )
                    # alternate eviction between DVE and ACT: keeps the DVE
                    # FIFO short so the SDPA-critical mask/normalize ops
                    # behind it aren't delayed by bulk projection copies
                    if tl % 2 == 0:
                        nc.vector.tensor_copy(st, pp)
                    else:
                        nc.scalar.copy(st, pp)
                    nc.sync.dma_start(
                        out=out_d[
                            tt * 128 : (tt + 1) * 128, nch * 512 : (nch + 1) * 512
                        ],
                        in_=st,
                    )

        for c in range(NCH):
            njt = 4 * (c + 1)  # causal: key tiles 0 .. 4c+3
            for hp in range(H_LOC // 2):
                # previous chunk's projection matmuls are drip-fed INTO the
                # jt loop below so the PE always has independent work while
                # the exp->mask->PV chain of the current tiles is in flight
                pending = list(range(4 * hp, 4 * hp + 4)) if c > 0 else []
                hh = (2 * hp, 2 * hp + 1)
                o_ps = {
                    h: psum.tile([128, 512], f32, name=f"o_ps{h}_{c}", tag="ps")
                    for h in hh
                }
                d_ps = {
                    h: psum.tile([128, 512], f32, name=f"d_ps{h}_{c}", tag="ps")
                    for h in hh
                }
                # interleave two heads so PE always has an independent matmul
                # while the scalar engine works on the other head's exp
                for jt in range(njt):
                    for h in hh:
                        s_ps = psum.tile(
                            [128, 512], f32, name=f"s_ps{h}_{c}_{jt}", tag="ps"
                        )
                        nc.tensor.matmul(
                            s_ps,
                            lhsT=qkT[:, 2 * h + 1, jt * 128 : (jt + 1) * 128],
                            rhs=qkT[:, 2 * h, c * 512 : (c + 1) * 512],
                            start=True,
                            stop=True,
                        )
                        att = attpool.tile(
                            [128, 512], bf, name=f"att{h}_{c}_{jt}", tag="att"
                        )
                        nc.scalar.activation(
                            out=att,
                            in_=s_ps,
                            func=mybir.ActivationFunctionType.Exp,
                            scale=SCALE,
                        )
                        r = jt - 4 * c
                        if r >= 0:
                            nc.vector.tensor_mul(att, att, masks[r])
                        nc.tensor.matmul(
                            d_ps[h],
                            lhsT=ones,
                            rhs=att,
                            start=(jt == 0),
                            stop=(jt == njt - 1),
                        )
                        nc.tensor.matmul(
                            o_ps[h],
                            lhsT=vN[:, jt, h * 128 : (h + 1) * 128],
                            rhs=att,
                            start=(jt == 0),
                            stop=(jt == njt - 1),
                        )
                    if pending and jt % 3 == 2:
                        proj_block(c - 1, [pending.pop(0)])
                for nch in pending:
                    proj_block(c - 1, [nch])
                for h in hh:
                    rec = recippool.tile([128, 512], f32, name=f"rec{h}_{c}", tag="rec")
                    nc.vector.reciprocal(rec, d_ps[h])
                    nc.vector.tensor_mul(yT[:, h, c * 512 : (c + 1) * 512], o_ps[h], rec)
        proj_block(NCH - 1, range(C // 512))

    for fr in reversed(frees):
        fr()


def _rope_tables():
    theta = 1.0 / (ROPE_BASE ** (np.arange(0, R, 2, dtype=np.float64) / R))  # (16,)
    ang = np.outer(np.arange(T, dtype=np.float64), theta)  # (T, 16)
    cos = np.cos(ang).T  # (16, T)
    sin = np.sin(ang).T
    cosP = np.concatenate([cos, cos], axis=0)  # (32, T)
    sinP = np.concatenate([-sin, sin], axis=0)
    return np.ascontiguousarray(cosP).astype(BF16), np.ascontiguousarray(sinP).astype(BF16)


def kernel(x, w_attn, b_attn, w_proj, b_proj):
    x = np.asarray(x, dtype=np.float32)
    w_attn = np.asarray(w_attn, dtype=np.float32)
    b_attn = np.asarray(b_attn, dtype=np.float32)
    w_proj = np.asarray(w_proj, dtype=np.float32)
    b_proj = np.asarray(b_proj, dtype=np.float32)
    B = x.shape[0]
    assert (B, x.shape[1], x.shape[2]) == (1, T, C)

    if "nc" not in _CACHE:
        _CACHE["nc"] = _build_program()
    nc = _CACHE["nc"]

    xT = np.ascontiguousarray(x[0].T).astype(BF16)  # (C, T)
    cosP, sinP = _rope_tables()
    # diagonal causal mask tiles: maskP[r, jj, ii] = 1.0 iff ii >= jj + 128*r
    jj = np.arange(128)[None, :, None]
    ii = np.arange(512)[None, None, :]
    rr = (128 * np.arange(4))[:, None, None]
    maskP = (ii >= jj + rr).astype(BF16)  # (4, 128, 512)

    # w_attn rows per head h: [q (128), k (128), v (128)] at offset h*384
    wa = w_attn.reshape(H, 3, D, C)
    ba = b_attn.reshape(H, 3, D)
    in_maps = []
    for core in range(N_CORES):
        hs = range(core * H_LOC, (core + 1) * H_LOC)
        qk_rows = np.concatenate(
            [wa[h, t] for h in hs for t in (0, 1)], axis=0
        )  # (1024, C)  order: q_h0, k_h0, q_h1, k_h1, ...
        v_rows = np.concatenate([wa[h, 2] for h in hs], axis=0)  # (512, C)
        wqkT = np.ascontiguousarray(qk_rows.T).astype(BF16)  # (C, 1024)
        wvT = np.ascontiguousarray(v_rows.T).astype(BF16)  # (C, 512)
        wpT = np.ascontiguousarray(
            w_proj[:, core * 512 : (core + 1) * 512].T
        ).astype(BF16)  # (512, C)
        bqk = np.ascontiguousarray(
            np.stack([ba[h, t] for h in hs for t in (0, 1)], axis=0).T
        ).astype(np.float32)  # (128, 8)
        in_maps.append(
            dict(
                xT=xT, wqkT=wqkT, wvT=wvT, wpT=wpT, bqk=bqk,
                cosP=cosP, sinP=sinP, maskP=maskP,
            )
        )

    res = bass_utils.run_bass_kernel_spmd(
        nc, in_maps, core_ids=list(range(N_CORES)), trace=TRACE
    )
    global LAST_EXEC_NS, LAST_RESULTS
    LAST_EXEC_NS = res.exec_time_ns
    LAST_RESULTS = res

    out = np.zeros((T, C), dtype=np.float32)
    for core in range(N_CORES):
        out += res.results[core]["out"]

    # bias folds: q/k biases were applied on device; the v bias adds exactly
    # b_v to every y row (softmax rows sum to 1), so it folds into the output
    # bias along with b_proj.
    b_v = ba[:, 2, :].reshape(-1)  # (4096,)
    out += (w_proj @ b_v + b_proj)[None, :]
    return out.reshape(B, T, C).astype(np.float32)
